# revision 1
# baseline (speedup 1.0000x reference)
"""Trainium2 Bass kernel for nn_NoConsolidationModel (scatter_memory).

Math: per batch element with window w = seqs[b, 55:63], query q:
    h   = relu(concat(embed[q], mean_j embed[w_j]) @ W1.T + b1)
    out = h @ W2.T + b2
Folding embed into layer 1 (linearity):
    E1a  = embed @ W1[:, :64].T          # [66, 64]
    E1bm = (embed @ W1[:, 64:].T) / 8    # [66, 64]
    h_pre = E1a.T @ onehot(q) + E1bm.T @ counts(w) + b1
so gather+mean+layer1 collapse into matmuls against count/one-hot vectors
(exact small ints, shipped as fp8e4m3; tables fp16 — PE accepts mixed).

Layout tricks (PE here is capped at 1 out-column per 1.2GHz cycle):
  - single-pass L1: u = [counts(66); onehot_q(62)] has K=128, valid when
    q < 62.  Each core's batch is permuted so q>=62 elements sit in the
    tail; a small second loop recomputes the last TAIL columns with the
    classic two-matmul (counts, full one-hot) form that works for any q.
  - even/odd 512-col slices share one [128, *] PSUM tile (partitions
    0-63 / 64-127) so ACT/DVE run at full width; two pairs share one
    [128, 1024] PSUM tile so ACT/DVE run one op per 2048 elements.
  - block-diagonal L2: lhsT = [[W2.T, 0], [0, W2.T]] computes both
    packed slices' logits in ONE matmul (two elements per PE column).

Sharding: pure data parallel, batch split across 8 cores; weights
replicated. Output stored as [64, n] f16 per core, transposed + upcast on
host.
"""

import sys

sys.path.insert(0, "/opt/trn_rl_repo")

import numpy as np
import ml_dtypes

B = 524288
NCORES = 8
V = 66          # VOCAB_SIZE + 2
H = 64          # HIDDEN_DIM
SEQ = 64
MEM = 8
WIN_LO = SEQ - 1 - MEM
WIN_HI = SEQ - 1
QSPLIT = 128 - V  # 62: queries below this go through the single-pass path

N_PER_CORE = B // NCORES
CH = 4096       # columns per DMA chunk (4 pairs = 2 groups)
TS = 512        # matmul slice width (one PSUM bank of f32)
TAIL = 5120     # tail columns recomputed by the any-q path

F8 = ml_dtypes.float8_e4m3

_PROG_CACHE = {}


def _build_program(n, ch, ts, tail):
    import concourse.tile as tile
    from concourse import bacc, mybir

    assert n % ch == 0 and ch % (4 * ts) == 0 and tail % (2 * ts) == 0
    f16 = mybir.dt.float16
    f32 = mybir.dt.float32
    f8 = mybir.dt.float8e4
    Relu = mybir.ActivationFunctionType.Relu

    nc = bacc.Bacc("TRN2", target_bir_lowering=False, debug=False,
                   num_devices=NCORES)

    u8 = mybir.dt.uint8
    u1_d = nc.dram_tensor("u1", [128, n], f8, kind="ExternalInput").ap()
    ohq2_d = nc.dram_tensor("ohq2", [V, tail], f8, kind="ExternalInput").ap()
    # all weights/biases packed in one blob: [t1 f16x64 | w2b f16x128 |
    # b1 f32 | b2 f32 | e1bm f16x64 | e1a f16x64] per partition row
    cb_d = nc.dram_tensor("cblob", [128, 648], u8, kind="ExternalInput").ap()
    out1_d = nc.dram_tensor("out1", [H, n], f16, kind="ExternalOutput").ap()
    out2_d = nc.dram_tensor("out2", [H, tail], f16, kind="ExternalOutput").ap()

    with tile.TileContext(nc) as tc:
        with (
            tc.tile_pool(name="const", bufs=1) as cpool,
            tc.tile_pool(name="uin", bufs=6) as u_pool,
            tc.tile_pool(name="tin", bufs=1) as t_pool,
            tc.tile_pool(name="hbuf", bufs=6) as h_pool,
            tc.tile_pool(name="obuf", bufs=4) as out_pool,
            tc.tile_pool(name="ph", bufs=2, space="PSUM") as ph_pool,
            tc.tile_pool(name="pl", bufs=2, space="PSUM") as pl_pool,
        ):
            # one DMA for every constant, bitcast views into the blob
            cb_t = cpool.tile([128, 648], u8)
            nc.sync.dma_start(cb_t[:], cb_d[:])
            t1_s = cb_t[:, 0:128].bitcast(f16)          # [128, 64]
            w2b_s = cb_t[:, 128:384].bitcast(f16)       # [128, 128]
            b1_s = cb_t[:, 384:388].bitcast(f32)        # [128, 1]
            b2_s = cb_t[:, 388:392].bitcast(f32)        # [128, 1]
            e1bm_s = cb_t[0:V, 392:520].bitcast(f16)    # [66, 64]
            e1a_s = cb_t[0:V, 520:648].bitcast(f16)     # [66, 64]

            def l2_and_out(ph, out_t, col0, width):
                # relu+bias, block-diag L2, +b2 with PSUM->SBUF f16 copy
                h_t = h_pool.tile([128, 2 * ts], f16, tag="h")
                nc.scalar.activation(h_t[:, :width], ph[:, :width], Relu,
                                     bias=b1_s)
                pl = pl_pool.tile([128, 2 * ts], f32, tag="pl")
                for s in range(width // ts):
                    nc.tensor.matmul(pl[:, s * ts:(s + 1) * ts],
                                     w2b_s, h_t[:, s * ts:(s + 1) * ts],
                                     start=True, stop=True)
                nc.vector.tensor_scalar_add(
                    out_t[:, col0:col0 + width], pl[:, :width], b2_s)

            def store(out_dram, c0, chunk, out_t):
                view = out_dram[:, c0:c0 + chunk].rearrange(
                    "r (pb two j) -> two r pb j", two=2, j=ts)
                nc.sync.dma_start(view[0], out_t[0:H, :])
                nc.sync.dma_start(view[1], out_t[H:128, :])

            # region 1: single-pass L1 (K=128), all n columns
            for c in range(n // ch):
                c0 = c * ch
                u_t = u_pool.tile([128, ch], f8, tag="u1")
                nc.scalar.dma_start(u_t[:], u1_d[:, c0:c0 + ch])
                out_t = out_pool.tile([128, ch // 2], f16, tag="o")
                for g in range(ch // (4 * ts)):   # group: 2 pairs = 4 slices
                    lo = g * 4 * ts
                    ph = ph_pool.tile([128, 2 * ts], f32, tag="ph")
                    for half in range(2):         # pair within group
                        a = lo + 2 * half * ts
                        po = half * ts
                        nc.tensor.matmul(ph[0:H, po:po + ts], t1_s,
                                         u_t[:, a:a + ts],
                                         start=True, stop=True)
                        nc.tensor.matmul(ph[H:128, po:po + ts], t1_s,
                                         u_t[:, a + ts:a + 2 * ts],
                                         start=True, stop=True)
                    l2_and_out(ph, out_t, g * 2 * ts, 2 * ts)
                store(out1_d, c0, ch, out_t)

            # region 2: recompute the tail with the any-q two-pass form
            cnt_t = t_pool.tile([V, tail], f8, tag="cnt2")
            nc.scalar.dma_start(cnt_t[:], u1_d[0:V, n - tail:n])
            ohq_t = t_pool.tile([V, tail], f8, tag="ohq2")
            nc.scalar.dma_start(ohq_t[:], ohq2_d[:])
            out_t2 = out_pool.tile([128, tail // 2], f16, tag="o2")
            for p in range(tail // (2 * ts)):
                lo = 2 * p * ts
                hi = lo + ts
                ph = ph_pool.tile([128, 2 * ts], f32, tag="ph")
                for col, a in ((slice(0, H), lo), (slice(H, 128), hi)):
                    nc.tensor.matmul(ph[col, 0:ts], e1bm_s,
                                     cnt_t[:, a:a + ts],
                                     start=True, stop=False)
                    nc.tensor.matmul(ph[col, 0:ts], e1a_s,
                                     ohq_t[:, a:a + ts],
                                     start=False, stop=True)
                l2_and_out(ph, out_t2, p * ts, ts)
            store(out2_d, 0, tail, out_t2)

    nc.compile()
    return nc


def _get_program(n, ch, ts, tail):
    key = (n, ch, ts, tail)
    if key not in _PROG_CACHE:
        _PROG_CACHE[key] = _build_program(n, ch, ts, tail)
    return _PROG_CACHE[key]


def _host_prep(seqs, query_tok, embed, W1, b1, W2, b2, n_cores, n, tail):
    embed = np.asarray(embed, dtype=np.float32)
    W1 = np.asarray(W1, dtype=np.float32)
    W2 = np.asarray(W2, dtype=np.float32)
    b1 = np.asarray(b1, dtype=np.float32)
    b2 = np.asarray(b2, dtype=np.float32)

    e1a = (embed @ W1[:, :H].T).astype(np.float16)            # [V, H]
    e1bm = ((embed @ W1[:, H:].T) / MEM).astype(np.float16)   # [V, H]
    t1 = np.concatenate([e1bm, e1a[:QSPLIT]], axis=0)         # [128, H]
    w2b = np.zeros((128, 128), dtype=np.float16)
    w2t = W2.T.astype(np.float16)
    w2b[:H, :H] = w2t
    w2b[H:, H:] = w2t
    b1x2 = np.concatenate([b1, b1]).reshape(128, 1).astype(np.float32)
    b2x2 = np.concatenate([b2, b2]).reshape(128, 1).astype(np.float32)
    pad = np.zeros((128 - V, H), dtype=np.float16)
    cblob = np.concatenate([
        t1.view(np.uint8), w2b.view(np.uint8),
        b1x2.view(np.uint8), b2x2.view(np.uint8),
        np.concatenate([e1bm, pad]).view(np.uint8),
        np.concatenate([e1a, pad]).view(np.uint8)], axis=1)   # [128, 648]

    win = np.ascontiguousarray(np.asarray(seqs)[:, WIN_LO:WIN_HI]).astype(
        np.int64, copy=False)                                  # [B', MEM]
    q = np.asarray(query_tok).astype(np.int64, copy=False)

    cols = np.arange(n, dtype=np.int64)
    in_maps = []
    perms = []
    for c in range(n_cores):
        w_c = win[c * n:(c + 1) * n]
        q_c = q[c * n:(c + 1) * n]
        hi_q = q_c >= QSPLIT
        n2 = int(hi_q.sum())
        assert n2 <= tail, f"core {c}: {n2} high-query elements > tail {tail}"
        perm = np.concatenate([np.flatnonzero(~hi_q), np.flatnonzero(hi_q)])
        perms.append(perm)
        wp = w_c[perm]
        qp = q_c[perm]
        u1 = np.zeros((128, n), dtype=np.uint8)
        flat = wp * n + cols[:, None]
        u1[:V] = np.bincount(flat.ravel(), minlength=V * n).astype(
            np.uint8).reshape(V, n)
        low = np.flatnonzero(qp < QSPLIT)
        u1[V + qp[low], low] = 1
        ohq2 = np.zeros((V, tail), dtype=np.uint8)
        ohq2[qp[n - tail:], np.arange(tail)] = 1
        in_maps.append({
            "u1": u1.astype(F8), "ohq2": ohq2.astype(F8), "cblob": cblob,
        })
    return in_maps, perms


def _assemble(results, perms, n, tail):
    out = np.empty((len(perms) * n, H), dtype=np.float32)
    for c, perm in enumerate(perms):
        o1 = results[c]["out1"].astype(np.float32).T      # [n, H] permuted
        o2 = results[c]["out2"].astype(np.float32).T      # [tail, H]
        o1[n - tail:] = o2
        out[c * n:(c + 1) * n][perm] = o1
    return out


def kernel(seqs, query_tok, embed, W1, b1, W2, b2):
    from concourse.bass_utils import run_bass_kernel_spmd

    n = N_PER_CORE
    in_maps, perms = _host_prep(seqs, query_tok, embed, W1, b1, W2, b2,
                                NCORES, n, TAIL)
    nc = _get_program(n, CH, TS, TAIL)
    res = run_bass_kernel_spmd(nc, in_maps, core_ids=list(range(NCORES)))
    return _assemble(res.results, perms, n, TAIL)

